# revision 1
# baseline (speedup 1.0000x reference)
"""GAT+GCN Trainium2 kernel: 8-core SPMD Bass/Tile implementation.

Sharding: nodes/graphs split contiguously across cores.  Edges assigned
to the core owning their dst node, sorted by dst, padded per 128-node
dst tile to K chunks of 128 edges.  Gather tables (h|a_src, g1) are
fp16 and AllGathered across cores.
"""
import numpy as np
import concourse.bass as bass
import concourse.bacc as bacc
import concourse.mybir as mybir
import concourse.tile as tile

f32 = mybir.dt.float32
f16 = mybir.dt.float16
i32 = mybir.dt.int32
AF = mybir.ActivationFunctionType
OP = mybir.AluOpType
AX = mybir.AxisListType

F = 78          # input feature dim
H = 10          # heads
HID = 780       # F*H
FW = HID + H    # htab row: h | a_src


def ceil_div(a, b):
    return (a + b - 1) // b


def host_prep(inp, n_cores=8):
    """Build per-core input maps + cfg from full inputs."""
    x = np.ascontiguousarray(np.asarray(inp["x"], np.float32))
    ei = np.asarray(inp["edge_index"], np.int64)
    tgt = np.asarray(inp["target"], np.int64)
    N = x.shape[0]
    B = tgt.shape[0]
    GN = N // B                # nodes per graph
    NS = N // n_cores
    T = NS // 128
    BL = B // n_cores

    loops = np.arange(N, dtype=np.int64)
    src = np.concatenate([ei[0], loops])
    dst = np.concatenate([ei[1], loops])
    E = src.shape[0]

    deg = np.bincount(dst, minlength=N).astype(np.float64)
    dinv = 1.0 / np.sqrt(deg)
    normv = (dinv[src] * dinv[dst]).astype(np.float32)

    order = np.argsort(dst, kind="stable")
    src_s = src[order].astype(np.int32)
    dst_s = dst[order].astype(np.int32)
    norm_s = normv[order]

    gtile = dst_s // 128
    n_gtiles = N // 128
    starts = np.searchsorted(gtile, np.arange(n_gtiles))
    cnts = np.searchsorted(gtile, np.arange(n_gtiles), side="right") - starts
    K = int(np.max(ceil_div(cnts, 128)))

    srcs_p = np.zeros((n_gtiles, 128, K), np.int32)
    dstf_p = np.full((n_gtiles, 128, K), 128.0, np.float32)
    norm_p = np.zeros((n_gtiles, 128, K), np.float32)
    j = np.arange(E) - starts[gtile]
    srcs_p[gtile, j % 128, j // 128] = src_s
    dstf_p[gtile, j % 128, j // 128] = (dst_s % 128).astype(np.float32)
    norm_p[gtile, j % 128, j // 128] = norm_s

    gat_w = np.asarray(inp["gat_w"], np.float32)
    att_src = np.asarray(inp["att_src"], np.float32)
    att_dst = np.asarray(inp["att_dst"], np.float32)
    As = np.einsum("fhc,hc->fh", gat_w.reshape(F, H, F), att_src)
    Ad = np.einsum("fhc,hc->fh", gat_w.reshape(F, H, F), att_dst)
    asad = np.concatenate([As, Ad], 1).astype(np.float32)
    gatb_rep = np.broadcast_to(np.asarray(inp["gat_b"], np.float32), (128, HID)).copy()
    gcn_w = np.asarray(inp["gcn_w"], np.float32)
    gcnb_rep = np.broadcast_to(np.asarray(inp["gcn_b"], np.float32), (128, HID)).copy()

    fcg1_w = np.asarray(inp["fcg1_w"], np.float32).copy()
    fcg1_w[HID:] *= 1.0 / GN

    def bias_sw(b, mt):
        b = np.asarray(b, np.float32)
        out = np.zeros((mt * 128,), np.float32)
        out[: b.shape[0]] = b
        return np.ascontiguousarray(out.reshape(mt, 128).T)

    fcg1_bsw = bias_sw(inp["fcg1_b"], 12)
    fcg2_w = np.asarray(inp["fcg2_w"], np.float32)
    fcg2_bsw = bias_sw(inp["fcg2_b"], 1)

    convxt_w = np.asarray(inp["convxt_w"], np.float32)
    W2 = np.ascontiguousarray(convxt_w.transpose(1, 2, 0).reshape(1000, 8 * 32))
    emb = np.asarray(inp["emb"], np.float32)
    fcxt_w = np.asarray(inp["fcxt_w"], np.float32)
    cb = np.asarray(inp["convxt_b"], np.float32)
    bias_fold = (cb[:, None] * fcxt_w.reshape(32, 121, 128).sum(1)).sum(0)
    fcxt_bsw = bias_sw(np.asarray(inp["fcxt_b"], np.float32) + bias_fold, 1)

    fc1_w = np.asarray(inp["fc1_w"], np.float32)
    fc1_bsw = bias_sw(inp["fc1_b"], 8)
    fc2_w = np.asarray(inp["fc2_w"], np.float32)
    fc2_bsw = bias_sw(inp["fc2_b"], 4)
    out_w = np.asarray(inp["out_w"], np.float32)
    out_b = np.asarray(inp["out_b"], np.float32).reshape(1, 1)

    iota_rep = np.broadcast_to(
        np.tile(np.arange(128, dtype=np.float32), K), (128, K * 128)).copy()
    iota26 = np.broadcast_to(
        np.tile(np.arange(26, dtype=np.float32), 8), (128, 8 * 26)).copy()
    ident = np.eye(128, dtype=np.float32)
    ident16 = np.eye(128, dtype=np.float16)
    ng = 128 // GN
    poolm = np.zeros((128, ng), np.float32)
    for g in range(ng):
        poolm[g * GN:(g + 1) * GN, g] = 1.0

    shared = dict(
        gat_w=gat_w, asad=asad, gatb_rep=gatb_rep, gcn_w=gcn_w,
        gcnb_rep=gcnb_rep, fcg1_w=fcg1_w, fcg1_bsw=fcg1_bsw, fcg2_w=fcg2_w,
        fcg2_bsw=fcg2_bsw, w2=W2, emb=emb, fcxt_w=fcxt_w, fcxt_bsw=fcxt_bsw,
        fc1_w=fc1_w, fc1_bsw=fc1_bsw, fc2_w=fc2_w, fc2_bsw=fc2_bsw,
        out_w=out_w, out_b=out_b, iota_rep=iota_rep, iota26=iota26,
        ident=ident, ident16=ident16, poolm=poolm,
    )

    in_maps = []
    for c in range(n_cores):
        m = dict(shared)
        m["x_sl"] = np.ascontiguousarray(x[c * NS:(c + 1) * NS])
        gt = slice(c * T, (c + 1) * T)
        m["srcs"] = np.ascontiguousarray(srcs_p[gt].reshape(T * 128, K))
        m["dstf"] = np.ascontiguousarray(dstf_p[gt].reshape(T * 128, K))
        m["normv"] = np.ascontiguousarray(norm_p[gt].reshape(T * 128, K))
        tpad = np.zeros((BL, 1024), np.int64)
        tpad[:, :1000] = tgt[c * BL:(c + 1) * BL]
        tl = tpad.reshape(BL, 8, 128)
        m["t_sb"] = np.ascontiguousarray(
            tl.transpose(2, 0, 1).reshape(128, BL * 8).astype(np.float32))
        in_maps.append(m)

    cfg = dict(n_cores=n_cores, N=N, NS=NS, T=T, BL=BL, K=K, GN=GN)
    return in_maps, cfg


def build(cfg, dbg=False, ablate=()):
    n_cores, NS, T, BL, K, GN = (cfg["n_cores"], cfg["NS"], cfg["T"],
                                 cfg["BL"], cfg["K"], cfg["GN"])
    N = cfg["N"]

    nc = bacc.Bacc(None, target_bir_lowering=False)

    def dinp(name, shape, dt=f32):
        return nc.dram_tensor(name, list(shape), dt, kind="ExternalInput")

    x_sl = dinp("x_sl", (NS, F))
    srcs = dinp("srcs", (T * 128, K), i32)
    dstf = dinp("dstf", (T * 128, K))
    normv = dinp("normv", (T * 128, K))
    t_sb_d = dinp("t_sb", (128, BL * 8))
    gat_w = dinp("gat_w", (F, HID))
    asad = dinp("asad", (F, 2 * H))
    gatb_rep = dinp("gatb_rep", (128, HID))
    gcn_w = dinp("gcn_w", (HID, HID))
    gcnb_rep = dinp("gcnb_rep", (128, HID))
    fcg1_w = dinp("fcg1_w", (2 * HID, 1500))
    fcg1_bsw = dinp("fcg1_bsw", (128, 12))
    fcg2_w = dinp("fcg2_w", (1500, 128))
    fcg2_bsw = dinp("fcg2_bsw", (128, 1))
    w2_d = dinp("w2", (1000, 256))
    emb_d = dinp("emb", (26, 128))
    fcxt_w = dinp("fcxt_w", (32 * 121, 128))
    fcxt_bsw = dinp("fcxt_bsw", (128, 1))
    fc1_w = dinp("fc1_w", (256, 1024))
    fc1_bsw = dinp("fc1_bsw", (128, 8))
    fc2_w = dinp("fc2_w", (1024, 512))
    fc2_bsw = dinp("fc2_bsw", (128, 4))
    out_w = dinp("out_w", (512, 1))
    out_b = dinp("out_b", (1, 1))
    iota_rep = dinp("iota_rep", (128, K * 128))
    iota26 = dinp("iota26", (128, 8 * 26))
    ident = dinp("ident", (128, 128))
    ident16 = dinp("ident16", (128, 128), f16)
    poolm = dinp("poolm", (128, 128 // GN))

    outp = nc.dram_tensor("outp", [1, BL], f32, kind="ExternalOutput")
    if dbg:
        o_htab = nc.dram_tensor("o_htab", [N, FW], f16, kind="ExternalOutput")
        o_g1 = nc.dram_tensor("o_g1", [NS, HID], f16, kind="ExternalOutput")
        o_g2 = nc.dram_tensor("o_g2", [NS, HID], f32, kind="ExternalOutput")
        o_gmpT = nc.dram_tensor("o_gmpT", [7 * 128, BL], f32, kind="ExternalOutput")
        o_gapT = nc.dram_tensor("o_gapT", [7 * 128, BL], f32, kind="ExternalOutput")
        o_xc0 = nc.dram_tensor("o_xc0", [128, BL], f32, kind="ExternalOutput")
        o_xc1 = nc.dram_tensor("o_xc1", [128, BL], f32, kind="ExternalOutput")
        o_y3 = nc.dram_tensor("o_y3", [128, 8 * BL], f32, kind="ExternalOutput")
        o_y4 = nc.dram_tensor("o_y4", [128, 4 * BL], f32, kind="ExternalOutput")
        o_osb = nc.dram_tensor("o_osb", [1, BL], f32, kind="ExternalOutput")

    htab_sl = nc.dram_tensor("htab_sl", [NS, FW], f16)
    htab = nc.dram_tensor("htab", [N, FW], f16, addr_space="Shared")
    g1_sl = nc.dram_tensor("g1_sl", [NS, HID], f16)
    gap_dram = nc.dram_tensor("gap_dram", [BL, HID], f32)
    g1tab = nc.dram_tensor("g1tab", [N, HID], f16, addr_space="Shared")

    FCH = [(kk * 128, min(128, HID - kk * 128)) for kk in range(ceil_div(HID, 128))]

    def tiles(n, step=128):
        return [(s, min(step, n - s)) for s in range(0, n, step)]

    with tile.TileContext(nc) as tc:
        with (
            tc.tile_pool(name="const", bufs=1) as cpool,
            tc.tile_pool(name="sb", bufs=2) as pool,
            tc.tile_pool(name="w", bufs=3) as wpool,
            tc.tile_pool(name="ps", bufs=2, space="PSUM") as psp,
            tc.tile_pool(name="pstr", bufs=2, space="PSUM") as pst,
            tc.tile_pool(name="psm", bufs=2, space="PSUM") as psm,
        ):
            # ---------- resident constants ----------
            def load_const(name, dram, shape, dt=f32, rows=None):
                t_ = cpool.tile(list(shape), dt, tag=name, name=name)
                if rows is None:
                    nc.sync.dma_start(out=t_[:], in_=dram[:])
                else:
                    nc.sync.dma_start(out=t_[:rows, :], in_=dram[:])
                return t_

            gatw_sb = load_const("gatw", gat_w, [F, HID])
            asad_sb = load_const("asad", asad, [F, 2 * H])
            gatb_sb = load_const("gatb", gatb_rep, [128, HID])
            gcnb_sb = load_const("gcnb", gcnb_rep, [128, HID])
            iota_sb = load_const("iota", iota_rep, [128, K * 128])
            iota26_sb = load_const("iota26", iota26, [128, 8 * 26])
            ident_sb = load_const("ident", ident, [128, 128])
            ident16_sb = load_const("ident16", ident16, [128, 128], f16)
            poolm_sb = load_const("poolm", poolm, [128, 2])
            emb_sb = load_const("emb", emb_d, [26, 128])
            t_sb = load_const("tsb", t_sb_d, [128, BL * 8])
            fcg1b_sb = load_const("fcg1b", fcg1_bsw, [128, 12])
            fcg2b_sb = load_const("fcg2b", fcg2_bsw, [128, 1])
            fcxtb_sb = load_const("fcxtb", fcxt_bsw, [128, 1])
            fc1b_sb = load_const("fc1b", fc1_bsw, [128, 8])
            fc2b_sb = load_const("fc2b", fc2_bsw, [128, 4])
            outb_sb = load_const("outb", out_b, [1, 1])

            gcnw_sb = []
            for kk, (ks, kn) in enumerate(FCH):
                t_ = cpool.tile([128, HID], f32, tag=f"gcnw{kk}", name=f"gcnw{kk}")
                nc.sync.dma_start(out=t_[:kn, :], in_=gcn_w[ks:ks + kn, :])
                gcnw_sb.append(t_)
            w2_sb = []
            for ic in range(8):
                icn = min(128, 1000 - ic * 128)
                t_ = cpool.tile([128, 256], f32, tag=f"w2{ic}", name=f"w2{ic}")
                nc.sync.dma_start(out=t_[:icn, :], in_=w2_d[ic * 128:ic * 128 + icn, :])
                w2_sb.append(t_)
            adst_sb = cpool.tile([128, T * H], f16, tag="adst")

            # ================= Phase A: h | a_src | a_dst ===============
            for t in range(T):
                rows = slice(t * 128, (t + 1) * 128)
                x_t = pool.tile([128, F], f32, tag="x_t")
                nc.sync.dma_start(out=x_t[:], in_=x_sl[rows, :])
                xt_ps = pst.tile([128, 128], f32, tag="tr")
                nc.tensor.transpose(out=xt_ps[:F, :], in_=x_t[:], identity=ident_sb[:])
                xT = pool.tile([F, 128], f32, tag="xT")
                nc.scalar.activation(out=xT[:], in_=xt_ps[:F, :], func=AF.Copy)
                h_ps = psp.tile([128, HID], f32, tag="big")
                nc.tensor.matmul(out=h_ps[:, :512], lhsT=xT[:], rhs=gatw_sb[:, :512],
                                 start=True, stop=True)
                nc.tensor.matmul(out=h_ps[:, 512:], lhsT=xT[:], rhs=gatw_sb[:, 512:],
                                 start=True, stop=True)
                asd_ps = psm.tile([128, 96], f32, tag="mlp")
                nc.tensor.matmul(out=asd_ps[:, :2 * H], lhsT=xT[:], rhs=asad_sb[:],
                                 start=True, stop=True)
                htile = pool.tile([128, HID], f16, tag="htile")
                nc.scalar.activation(out=htile[:], in_=h_ps[:], func=AF.Copy)
                asd16 = pool.tile([128, 2 * H], f16, tag="asd")
                nc.vector.tensor_copy(out=asd16[:], in_=asd_ps[:, :2 * H])
                nc.sync.dma_start(out=htab_sl[rows, :HID], in_=htile[:])
                nc.sync.dma_start(out=htab_sl[rows, HID:FW], in_=asd16[:, :H])
                nc.vector.tensor_copy(out=adst_sb[:, t * H:(t + 1) * H],
                                      in_=asd16[:, H:])

            if "coll" not in ablate:
                nc.gpsimd.collective_compute(
                    "AllGather", OP.bypass,
                    replica_groups=[list(range(n_cores))],
                    ins=[htab_sl[:]], outs=[htab[:]],
                )
            else:
                nc.gpsimd.dma_start(out=htab[:NS, :], in_=htab_sl[:])

            # ================= Phase B: GAT edge aggregation =============
            for t in range(T):
                rows = slice(t * 128, (t + 1) * 128)
                sc = pool.tile([128, K], i32, tag="sc")
                nc.sync.dma_start(out=sc[:], in_=srcs[rows, :])
                df = pool.tile([128, K], f32, tag="df")
                nc.sync.dma_start(out=df[:], in_=dstf[rows, :])
                G = pool.tile([128, K * FW], f16, tag="G")
                for c in range(K):
                    if "gather" in ablate:
                        nc.sync.dma_start(
                            out=G[:, c * FW:(c + 1) * FW],
                            in_=htab[t * 128:(t + 1) * 128, :])
                    else:
                        nc.gpsimd.indirect_dma_start(
                            out=G[:, c * FW:(c + 1) * FW], out_offset=None,
                            in_=htab[:],
                            in_offset=bass.IndirectOffsetOnAxis(ap=sc[:, c:c + 1], axis=0),
                        )
                sel = pool.tile([128, K * 128], f16, tag="sel")
                nc.vector.tensor_tensor(
                    out=sel[:].rearrange("p (k d) -> p k d", d=128),
                    in0=iota_sb[:].rearrange("p (k d) -> p k d", d=128),
                    in1=df[:].unsqueeze(2).to_broadcast([128, K, 128]),
                    op=OP.is_equal)
                sm_ps = psm.tile([128, 96], f32, tag="mlp")
                for c in range(K):
                    st_ps = pst.tile([128, 128], f16, tag="tr")
                    nc.tensor.transpose(out=st_ps[:], in_=sel[:, c * 128:(c + 1) * 128],
                                        identity=ident16_sb[:])
                    selT = pool.tile([128, 128], f16, tag=f"selT{c}")
                    nc.scalar.activation(out=selT[:], in_=st_ps[:], func=AF.Copy)
                    nc.tensor.matmul(out=sm_ps[:, c * H:(c + 1) * H], lhsT=selT[:],
                                     rhs=adst_sb[:, t * H:(t + 1) * H],
                                     start=True, stop=True)
                al = pool.tile([128, K * H], f32, tag="al")
                nc.vector.tensor_tensor(
                    out=al[:].rearrange("p (k h) -> p k h", h=H),
                    in0=G[:].rearrange("p (k w) -> p k w", w=FW)[:, :, HID:FW],
                    in1=sm_ps[:, :K * H].rearrange("p (k h) -> p k h", h=H),
                    op=OP.add)
                al2 = pool.tile([128, K * H], f32, tag="al2")
                nc.vector.tensor_scalar(out=al2[:], in0=al[:], scalar1=0.2,
                                        scalar2=None, op0=OP.mult)
                nc.vector.tensor_tensor(out=al2[:], in0=al2[:], in1=al[:], op=OP.max)
                p16 = pool.tile([128, K * H], f16, tag="p16")
                nc.scalar.activation(out=p16[:], in_=al2[:], func=AF.Exp)
                for c in range(K):
                    nc.tensor.matmul(out=sm_ps[:, 80:80 + H],
                                     lhsT=sel[:, c * 128:(c + 1) * 128],
                                     rhs=p16[:, c * H:(c + 1) * H],
                                     start=(c == 0), stop=(c == K - 1))
                m = pool.tile([128, K * HID], f16, tag="m")
                nc.vector.tensor_tensor(
                    out=m[:].rearrange("p (k h r) -> p k h r", h=H, r=F),
                    in0=G[:].rearrange("p (k w) -> p k w", w=FW)[:, :, :HID]
                         .rearrange("p k (h r) -> p k h r", r=F),
                    in1=p16[:].rearrange("p (k h) -> p k h", h=H)
                         .unsqueeze(3).to_broadcast([128, K, H, F]),
                    op=OP.mult)
                g1_ps = psp.tile([128, HID], f32, tag="big")
                for c in range(K):
                    nc.tensor.matmul(out=g1_ps[:, :512],
                                     lhsT=sel[:, c * 128:(c + 1) * 128],
                                     rhs=m[:, c * HID: c * HID + 512],
                                     start=(c == 0), stop=(c == K - 1))
                    nc.tensor.matmul(out=g1_ps[:, 512:],
                                     lhsT=sel[:, c * 128:(c + 1) * 128],
                                     rhs=m[:, c * HID + 512:(c + 1) * HID],
                                     start=(c == 0), stop=(c == K - 1))
                rd = pool.tile([128, H], f32, tag="rd")
                nc.vector.reciprocal(out=rd[:], in_=sm_ps[:, 80:80 + H])
                g1n = pool.tile([128, HID], f32, tag="g1n")
                nc.vector.tensor_tensor(
                    out=g1n[:].rearrange("p (h r) -> p h r", r=F),
                    in0=g1_ps[:].rearrange("p (h r) -> p h r", r=F),
                    in1=rd[:].unsqueeze(2).to_broadcast([128, H, F]),
                    op=OP.mult)
                nc.vector.tensor_tensor(out=g1n[:], in0=g1n[:], in1=gatb_sb[:],
                                        op=OP.add)
                g1t = pool.tile([128, HID], f16, tag="g1t")
                nc.scalar.activation(out=g1t[:], in_=g1n[:], func=AF.Relu)
                nc.sync.dma_start(out=g1_sl[rows, :], in_=g1t[:])

            if "coll" not in ablate:
                nc.gpsimd.collective_compute(
                    "AllGather", OP.bypass,
                    replica_groups=[list(range(n_cores))],
                    ins=[g1_sl[:]], outs=[g1tab[:]],
                )
            else:
                nc.gpsimd.dma_start(out=g1tab[:NS, :], in_=g1_sl[:])

            # ============ Phase D: GCN aggregation + z + pooling =========
            gmpT_sb = []
            gapT_sb = []
            for kk, (ks, kn) in enumerate(FCH):
                gmpT_sb.append(cpool.tile([128, BL], f32, tag=f"gmpT{kk}", name=f"gmpT{kk}"))
                gapT_sb.append(cpool.tile([128, BL], f32, tag=f"gapT{kk}", name=f"gapT{kk}"))

            for t in range(T):
                rows = slice(t * 128, (t + 1) * 128)
                sc = pool.tile([128, K], i32, tag="sc")
                nc.sync.dma_start(out=sc[:], in_=srcs[rows, :])
                df = pool.tile([128, K], f32, tag="df")
                nc.sync.dma_start(out=df[:], in_=dstf[rows, :])
                nv = pool.tile([128, K], f32, tag="nv")
                nc.sync.dma_start(out=nv[:], in_=normv[rows, :])
                G2 = pool.tile([128, K * HID], f16, tag="G")
                for c in range(K):
                    if "gather" in ablate:
                        nc.sync.dma_start(
                            out=G2[:, c * HID:(c + 1) * HID],
                            in_=g1tab[t * 128:(t + 1) * 128, :])
                    else:
                        nc.gpsimd.indirect_dma_start(
                            out=G2[:, c * HID:(c + 1) * HID], out_offset=None,
                            in_=g1tab[:],
                            in_offset=bass.IndirectOffsetOnAxis(ap=sc[:, c:c + 1], axis=0),
                        )
                sel = pool.tile([128, K * 128], f16, tag="sel")
                nc.vector.tensor_tensor(
                    out=sel[:].rearrange("p (k d) -> p k d", d=128),
                    in0=iota_sb[:].rearrange("p (k d) -> p k d", d=128),
                    in1=df[:].unsqueeze(2).to_broadcast([128, K, 128]),
                    op=OP.is_equal)
                wsel = pool.tile([128, K * 128], f16, tag="wsel")
                nc.vector.tensor_tensor(
                    out=wsel[:].rearrange("p (k d) -> p k d", d=128),
                    in0=sel[:].rearrange("p (k d) -> p k d", d=128),
                    in1=nv[:].unsqueeze(2).to_broadcast([128, K, 128]),
                    op=OP.mult)
                agg_ps = psp.tile([128, HID], f32, tag="big")
                for c in range(K):
                    nc.tensor.matmul(out=agg_ps[:, :512],
                                     lhsT=wsel[:, c * 128:(c + 1) * 128],
                                     rhs=G2[:, c * HID: c * HID + 512],
                                     start=(c == 0), stop=(c == K - 1))
                    nc.tensor.matmul(out=agg_ps[:, 512:],
                                     lhsT=wsel[:, c * 128:(c + 1) * 128],
                                     rhs=G2[:, c * HID + 512:(c + 1) * HID],
                                     start=(c == 0), stop=(c == K - 1))
                s_sb = pool.tile([128, HID], f32, tag="s_sb")
                nc.scalar.activation(out=s_sb[:], in_=agg_ps[:], func=AF.Copy)
                z_ps = psp.tile([128, HID], f32, tag="big")
                for kk, (ks, kn) in enumerate(FCH):
                    sT_ps = pst.tile([128, 128], f32, tag="tr")
                    nc.tensor.transpose(out=sT_ps[:kn, :], in_=s_sb[:, ks:ks + kn],
                                        identity=ident_sb[:])
                    sT = pool.tile([128, 128], f32, tag=f"sT{kk}")
                    nc.scalar.activation(out=sT[:kn, :], in_=sT_ps[:kn, :], func=AF.Copy)
                    nc.tensor.matmul(out=z_ps[:, :512], lhsT=sT[:kn, :],
                                     rhs=gcnw_sb[kk][:kn, :512],
                                     start=(kk == 0), stop=(kk == len(FCH) - 1))
                    nc.tensor.matmul(out=z_ps[:, 512:], lhsT=sT[:kn, :],
                                     rhs=gcnw_sb[kk][:kn, 512:],
                                     start=(kk == 0), stop=(kk == len(FCH) - 1))
                g2a = pool.tile([128, HID], f32, tag="g2a")
                nc.vector.tensor_tensor(out=g2a[:], in0=z_ps[:], in1=gcnb_sb[:],
                                        op=OP.add)
                g2b = pool.tile([128, HID], f32, tag="g2b")
                nc.scalar.activation(out=g2b[:], in_=g2a[:], func=AF.Relu)
                if dbg:
                    nc.sync.dma_start(out=o_g2[rows, :], in_=g2b[:])
                ng = 128 // GN      # graphs per tile
                gp_ps = psp.tile([128, HID], f32, tag="big", name="gp_ps")
                nc.tensor.matmul(out=gp_ps[:ng, :512],
                                 lhsT=poolm_sb[:, :ng],
                                 rhs=g2b[:, :512], start=True, stop=True)
                nc.tensor.matmul(out=gp_ps[:ng, 512:],
                                 lhsT=poolm_sb[:, :ng],
                                 rhs=g2b[:, 512:], start=True, stop=True)
                gtmp = pool.tile([128 // GN, HID], f32, tag="gtmp")
                nc.scalar.activation(out=gtmp[:], in_=gp_ps[:ng, :], func=AF.Copy)
                nc.sync.dma_start(out=gap_dram[ng * t:ng * (t + 1), :], in_=gtmp[:])
                for kk, (ks, kn) in enumerate(FCH):
                    tp_ps = pst.tile([128, 128], f32, tag="tr")
                    nc.tensor.transpose(out=tp_ps[:kn, :], in_=g2b[:, ks:ks + kn],
                                        identity=ident_sb[:])
                    nc.vector.reduce_max(
                        out=gmpT_sb[kk][:kn, ng * t:ng * (t + 1)],
                        in_=tp_ps[:kn, :].rearrange("p (g n) -> p g n", n=GN),
                        axis=AX.X)

            gap_acc = cpool.tile([BL, HID], f32, tag="gap_acc")
            nc.sync.dma_start(out=gap_acc[:], in_=gap_dram[:])
            for kk, (ks, kn) in enumerate(FCH):
                gt_ps = pst.tile([128, 128], f32, tag="tr", name="gt_ps")
                nc.tensor.transpose(out=gt_ps[:kn, :BL], in_=gap_acc[:, ks:ks + kn],
                                    identity=ident_sb[:BL, :BL])
                nc.scalar.activation(out=gapT_sb[kk][:kn, :], in_=gt_ps[:kn, :BL],
                                     func=AF.Copy)

            if dbg:
                nc.gpsimd.dma_start(out=o_htab[:], in_=htab[:])
                nc.gpsimd.dma_start(out=o_g1[:], in_=g1_sl[:])
                for kk in range(len(FCH)):
                    nc.sync.dma_start(out=o_gmpT[kk * 128:kk * 128 + 128, :],
                                      in_=gmpT_sb[kk][:])
                    nc.sync.dma_start(out=o_gapT[kk * 128:kk * 128 + 128, :],
                                      in_=gapT_sb[kk][:])

            # ================= Phase E: MLPs =================
            y1_sb = cpool.tile([128, 12 * BL], f32, tag="y1")
            for mi, (ms, mn) in enumerate(tiles(1500)):
                y_ps = psm.tile([128, BL], f32, tag="mlp")
                for kk, (ks, kn) in enumerate(FCH):
                    wt = wpool.tile([128, 128], f32, tag="wt")
                    nc.sync.dma_start(out=wt[:kn, :mn],
                                      in_=fcg1_w[ks:ks + kn, ms:ms + mn])
                    nc.tensor.matmul(out=y_ps[:mn, :], lhsT=wt[:kn, :mn],
                                     rhs=gmpT_sb[kk][:kn, :],
                                     start=(kk == 0), stop=False)
                for kk, (ks, kn) in enumerate(FCH):
                    wt = wpool.tile([128, 128], f32, tag="wt")
                    nc.sync.dma_start(out=wt[:kn, :mn],
                                      in_=fcg1_w[HID + ks:HID + ks + kn, ms:ms + mn])
                    nc.tensor.matmul(out=y_ps[:mn, :], lhsT=wt[:kn, :mn],
                                     rhs=gapT_sb[kk][:kn, :],
                                     start=False, stop=(kk == len(FCH) - 1))
                nc.scalar.activation(out=y1_sb[:mn, mi * BL:(mi + 1) * BL],
                                     in_=y_ps[:mn, :], func=AF.Relu,
                                     bias=fcg1b_sb[:mn, mi:mi + 1])

            xc0 = cpool.tile([128, BL], f32, tag="xc0")
            y2_ps = psm.tile([128, BL], f32, tag="mlp")
            kt2 = tiles(1500)
            for kk, (ks, kn) in enumerate(kt2):
                wt = wpool.tile([128, 128], f32, tag="wt")
                nc.sync.dma_start(out=wt[:kn, :], in_=fcg2_w[ks:ks + kn, :])
                nc.tensor.matmul(out=y2_ps[:], lhsT=wt[:kn, :],
                                 rhs=y1_sb[:kn, kk * BL:(kk + 1) * BL],
                                 start=(kk == 0), stop=(kk == len(kt2) - 1))
            nc.vector.tensor_scalar(out=xc0[:], in0=y2_ps[:],
                                    scalar1=fcg2b_sb[:, 0:1], scalar2=None,
                                    op0=OP.add)

            # ---- protein branch ----
            cvT_sb = cpool.tile([121, BL * 32], f32, tag="cvT")
            conv_bl = range(0) if "conv" in ablate else range(BL)
            if "conv" in ablate:
                nc.gpsimd.memset(cvT_sb[:], 0.0)
            for b in conv_bl:
                oh = pool.tile([128, 8 * 26], f32, tag="oh")
                nc.vector.tensor_tensor(
                    out=oh[:].rearrange("p (k c) -> p k c", c=26),
                    in0=iota26_sb[:].rearrange("p (k c) -> p k c", c=26),
                    in1=t_sb[:, b * 8:(b + 1) * 8].unsqueeze(2)
                        .to_broadcast([128, 8, 26]),
                    op=OP.is_equal)
                at_ps = psm.tile([26, 256], f32, tag="mlp")
                for ic in range(8):
                    icn = min(128, 1000 - ic * 128)
                    nc.tensor.matmul(out=at_ps[:],
                                     lhsT=oh[:icn, ic * 26:(ic + 1) * 26],
                                     rhs=w2_sb[ic][:icn, :],
                                     start=(ic == 0), stop=(ic == 7))
                at_sb = pool.tile([26, 256], f32, tag="at_sb")
                nc.scalar.activation(out=at_sb[:], in_=at_ps[:], func=AF.Copy)
                cv_ps = psm.tile([121, 32], f32, tag="mlp")
                for k in range(8):
                    nc.tensor.matmul(out=cv_ps[:], lhsT=emb_sb[:, k:k + 121],
                                     rhs=at_sb[:, k * 32:(k + 1) * 32],
                                     start=(k == 0), stop=(k == 7))
                nc.scalar.activation(out=cvT_sb[:, b * 32:(b + 1) * 32], in_=cv_ps[:],
                                     func=AF.Copy)
            xc1 = cpool.tile([128, BL], f32, tag="xc1")
            xt_ps = psm.tile([128, BL], f32, tag="mlp")
            for o in range(32):
                wt = wpool.tile([128, 128], f32, tag="wt")
                nc.sync.dma_start(out=wt[:121, :], in_=fcxt_w[o * 121:(o + 1) * 121, :])
                nc.tensor.matmul(
                    out=xt_ps[:], lhsT=wt[:121, :],
                    rhs=cvT_sb[:].rearrange("p (b o) -> p b o", o=32)[:, :, o],
                    start=(o == 0), stop=(o == 31))
            nc.vector.tensor_scalar(out=xc1[:], in0=xt_ps[:],
                                    scalar1=fcxtb_sb[:, 0:1], scalar2=None,
                                    op0=OP.add)

            if dbg:
                nc.sync.dma_start(out=o_xc0[:], in_=xc0[:])
                nc.sync.dma_start(out=o_xc1[:], in_=xc1[:])

            # ---- head ----
            y3_sb = cpool.tile([128, 8 * BL], f32, tag="y3")
            for mi in range(8):
                y_ps = psm.tile([128, BL], f32, tag="mlp")
                for kk in range(2):
                    wt = wpool.tile([128, 128], f32, tag="wt")
                    nc.sync.dma_start(out=wt[:],
                                      in_=fc1_w[kk * 128:(kk + 1) * 128,
                                                mi * 128:(mi + 1) * 128])
                    rhs = xc0 if kk == 0 else xc1
                    nc.tensor.matmul(out=y_ps[:], lhsT=wt[:], rhs=rhs[:],
                                     start=(kk == 0), stop=(kk == 1))
                nc.scalar.activation(out=y3_sb[:, mi * BL:(mi + 1) * BL], in_=y_ps[:],
                                     func=AF.Relu, bias=fc1b_sb[:, mi:mi + 1])
            y4_sb = cpool.tile([128, 4 * BL], f32, tag="y4")
            for mi in range(4):
                y_ps = psm.tile([128, BL], f32, tag="mlp")
                for kk in range(8):
                    wt = wpool.tile([128, 128], f32, tag="wt")
                    nc.sync.dma_start(out=wt[:],
                                      in_=fc2_w[kk * 128:(kk + 1) * 128,
                                                mi * 128:(mi + 1) * 128])
                    nc.tensor.matmul(out=y_ps[:], lhsT=wt[:],
                                     rhs=y3_sb[:, kk * BL:(kk + 1) * BL],
                                     start=(kk == 0), stop=(kk == 7))
                nc.scalar.activation(out=y4_sb[:, mi * BL:(mi + 1) * BL], in_=y_ps[:],
                                     func=AF.Relu, bias=fc2b_sb[:, mi:mi + 1])
            if dbg:
                nc.sync.dma_start(out=o_y3[:], in_=y3_sb[:])
                nc.sync.dma_start(out=o_y4[:], in_=y4_sb[:])
            o_ps = psm.tile([1, BL], f32, tag="mlp")
            for kk in range(4):
                wt = wpool.tile([128, 1], f32, tag="wto")
                nc.sync.dma_start(out=wt[:], in_=out_w[kk * 128:(kk + 1) * 128, :])
                nc.tensor.matmul(out=o_ps[:], lhsT=wt[:],
                                 rhs=y4_sb[:, kk * BL:(kk + 1) * BL],
                                 start=(kk == 0), stop=(kk == 3))
            o_sb = cpool.tile([1, BL], f32, tag="o_sb")
            nc.vector.tensor_scalar(out=o_sb[:], in0=o_ps[:],
                                    scalar1=outb_sb[:, 0:1], scalar2=None,
                                    op0=OP.add)
            if dbg:
                nc.sync.dma_start(out=o_osb[:], in_=o_sb[:])
            nc.sync.dma_start(out=outp[:], in_=o_sb[:])

    nc.finalize()
    return nc


def run(inp, n_cores=8, trace=False, dbg=False):
    from concourse.bass_utils import run_bass_kernel_spmd
    in_maps, cfg = host_prep(inp, n_cores)
    nc = build(cfg, dbg=dbg)
    res = run_bass_kernel_spmd(
        nc, in_maps, list(range(n_cores)), trace=trace,
        trace_cores=list(range(n_cores)) if trace else None)
    out = np.concatenate(
        [res.results[c]["outp"].reshape(-1, 1) for c in range(n_cores)], 0)
    return out, res


_CACHED = {}


def kernel(**inputs):
    """Full-input entry point: shards across 8 NeuronCores internally."""
    n_cores = 8
    in_maps, cfg = host_prep(inputs, n_cores)
    key = (cfg["N"], cfg["T"], cfg["BL"], cfg["K"], cfg["GN"])
    nc = _CACHED.get(key)
    if nc is None:
        nc = build(cfg)
        _CACHED[key] = nc
    from concourse.bass_utils import run_bass_kernel_spmd
    res = run_bass_kernel_spmd(nc, in_maps, list(range(n_cores)))
    out = np.concatenate(
        [res.results[c]["outp"].reshape(-1, 1) for c in range(n_cores)], 0)
    return out.astype(np.float32)



# revision 3
# speedup vs baseline: 5.4454x; 5.4454x over previous
"""GAT+GCN Trainium2 kernel: 8-core SPMD Bass/Tile implementation.

Sharding: nodes/graphs split contiguously across cores.  Edges (without
self loops) assigned to the core owning their dst node, sorted by dst,
padded per 128-node dst tile to K chunks of 128 edges; self-loop terms
are applied analytically (identity selection / per-partition scalars).
Gather tables (h|a_src, g1) are fp8-e4m3 and AllGathered across cores.
Dense matmuls run in fp16; MLP weights are prefetched to SBUF early so
phase E is compute-only.  The protein-branch conv runs split in the
shadow of the two AllGathers.
"""
import numpy as np
import concourse.bass as bass
import concourse.bacc as bacc
import concourse.mybir as mybir
import concourse.tile as tile

f32 = mybir.dt.float32
f16 = mybir.dt.float16
f8 = mybir.dt.float8e4
i32 = mybir.dt.int32
AF = mybir.ActivationFunctionType
OP = mybir.AluOpType
AX = mybir.AxisListType

F = 78          # input feature dim
H = 10          # heads
HID = 780       # F*H
FW = HID + H    # htab row: h | a_src
NKK = 7         # ceil(HID/128)


def ceil_div(a, b):
    return (a + b - 1) // b


def _chunk_rows(w, n_chunks, rows_per=128):
    """Pack w[rows, cols] row-chunks into [128, n_chunks*cols] lhsT tiles."""
    rows, cols = w.shape
    out = np.zeros((n_chunks, rows_per, cols), np.float32)
    for j in range(n_chunks):
        ks = j * rows_per
        kn = min(rows_per, rows - ks)
        if kn > 0:
            out[j, :kn] = w[ks:ks + kn]
    return np.ascontiguousarray(
        out.transpose(1, 0, 2).reshape(rows_per, n_chunks * cols))


def host_prep(inp, n_cores=8):
    """Build per-core input maps + cfg from full inputs."""
    x = np.asarray(inp["x"], np.float32)
    ei = np.asarray(inp["edge_index"], np.int64)
    tgt = np.asarray(inp["target"], np.int64)
    N = x.shape[0]
    B = tgt.shape[0]
    GN = N // B                # nodes per graph
    NS = N // n_cores
    T = NS // 128
    BL = B // n_cores

    src = ei[0]
    dst = ei[1]
    E = src.shape[0]

    # degrees include the self loops the reference adds
    deg = np.bincount(dst, minlength=N).astype(np.float64) + 1.0
    dinv = 1.0 / np.sqrt(deg)
    normv = (dinv[src] * dinv[dst]).astype(np.float32)
    nself = (dinv * dinv).astype(np.float32)

    order = np.argsort(dst, kind="stable")
    src_s = src[order].astype(np.int32)
    dst_s = dst[order].astype(np.int32)
    norm_s = normv[order]

    gtile = dst_s // 128
    n_gtiles = N // 128
    starts = np.searchsorted(gtile, np.arange(n_gtiles))
    cnts = np.searchsorted(gtile, np.arange(n_gtiles), side="right") - starts
    K = int(np.max(ceil_div(cnts, 128)))

    srcs_p = np.zeros((n_gtiles, 128, K), np.int32)
    dstf_p = np.full((n_gtiles, 128, K), 128.0, np.float16)
    norm_p = np.zeros((n_gtiles, 128, K), np.float32)
    j = np.arange(E) - starts[gtile]
    srcs_p[gtile, j % 128, j // 128] = src_s
    dstf_p[gtile, j % 128, j // 128] = (dst_s % 128).astype(np.float16)
    norm_p[gtile, j % 128, j // 128] = norm_s

    gat_w = np.asarray(inp["gat_w"], np.float32)
    att_src = np.asarray(inp["att_src"], np.float32)
    att_dst = np.asarray(inp["att_dst"], np.float32)
    As = np.einsum("fhc,hc->fh", gat_w.reshape(F, H, F), att_src)
    Ad = np.einsum("fhc,hc->fh", gat_w.reshape(F, H, F), att_dst)
    asad16 = np.concatenate([As, Ad], 1).astype(np.float16)
    gatw16 = gat_w.astype(np.float16)
    gatb_rep = np.broadcast_to(
        np.asarray(inp["gat_b"], np.float32), (128, HID)).copy()
    gcn_w = np.asarray(inp["gcn_w"], np.float32)
    gcnb_rep = np.broadcast_to(
        np.asarray(inp["gcn_b"], np.float32), (128, HID)).copy()
    gcnw16 = _chunk_rows(gcn_w, NKK).astype(np.float16)

    fcg1_w = np.asarray(inp["fcg1_w"], np.float32).copy()
    fcg1_w[HID:] *= 1.0 / GN
    # 14 chunks: 7 for the gmp rows (0:780), 7 for the gap rows (780:1560)
    f14 = np.zeros((14, 128, 1500), np.float32)
    for jj in range(NKK):
        ks = jj * 128
        kn = min(128, HID - ks)
        f14[jj, :kn] = fcg1_w[ks:ks + kn]
        f14[NKK + jj, :kn] = fcg1_w[HID + ks:HID + ks + kn]
    fcg1w16 = np.ascontiguousarray(
        f14.transpose(1, 0, 2).reshape(128, 14 * 1500)).astype(np.float16)

    def bias_sw(b, mt):
        b = np.asarray(b, np.float32)
        out = np.zeros((mt * 128,), np.float32)
        out[: b.shape[0]] = b
        return np.ascontiguousarray(out.reshape(mt, 128).T)

    fcg1_bsw = bias_sw(inp["fcg1_b"], 12)
    fcg2_w = np.asarray(inp["fcg2_w"], np.float32)
    fcg2w16 = _chunk_rows(fcg2_w, 12).astype(np.float16)
    fcg2_bsw = bias_sw(inp["fcg2_b"], 1)

    convxt_w = np.asarray(inp["convxt_w"], np.float32)
    W2 = np.ascontiguousarray(convxt_w.transpose(1, 2, 0).reshape(1000, 8 * 32))
    w2_16 = _chunk_rows(W2, 8).astype(np.float16)
    emb16 = np.asarray(inp["emb"], np.float32).astype(np.float16)
    fcxt_w = np.asarray(inp["fcxt_w"], np.float32)
    fxp = np.zeros((128, 32 * 128), np.float32)
    fxp[:121] = fcxt_w.reshape(32, 121, 128).transpose(1, 0, 2).reshape(
        121, 32 * 128)
    fcxtw16 = fxp.astype(np.float16)
    cb = np.asarray(inp["convxt_b"], np.float32)
    bias_fold = (cb[:, None] * fcxt_w.reshape(32, 121, 128).sum(1)).sum(0)
    fcxt_bsw = bias_sw(np.asarray(inp["fcxt_b"], np.float32) + bias_fold, 1)

    fc1_w = np.asarray(inp["fc1_w"], np.float32)
    fc1w16 = _chunk_rows(fc1_w, 2).astype(np.float16)
    fc1_bsw = bias_sw(inp["fc1_b"], 8)
    fc2_w = np.asarray(inp["fc2_w"], np.float32)
    fc2w16 = _chunk_rows(fc2_w, 8).astype(np.float16)
    fc2_bsw = bias_sw(inp["fc2_b"], 4)
    out_w = np.asarray(inp["out_w"], np.float32)
    outw16 = np.ascontiguousarray(
        out_w.reshape(4, 128).T).astype(np.float16)
    out_b = np.asarray(inp["out_b"], np.float32).reshape(1, 1)

    iota16 = np.broadcast_to(
        np.tile(np.arange(128, dtype=np.float16), K), (128, K * 128)).copy()
    iota26 = np.broadcast_to(
        np.tile(np.arange(26, dtype=np.float16), 8), (128, 8 * 26)).copy()
    ident16 = np.eye(128, dtype=np.float16)
    ng = 128 // GN
    poolm16 = np.zeros((128, ng), np.float16)
    for g in range(ng):
        poolm16[g * GN:(g + 1) * GN, g] = 1.0

    shared = dict(
        gatw=gatw16, asad=asad16, gatb_rep=gatb_rep, gcnw=gcnw16,
        gcnb_rep=gcnb_rep, fcg1w=fcg1w16, fcg1_bsw=fcg1_bsw, fcg2w=fcg2w16,
        fcg2_bsw=fcg2_bsw, w2=w2_16, emb=emb16, fcxtw=fcxtw16,
        fcxt_bsw=fcxt_bsw, fc1w=fc1w16, fc1_bsw=fc1_bsw, fc2w=fc2w16,
        fc2_bsw=fc2_bsw, outw=outw16, out_b=out_b, iota=iota16,
        iota26=iota26, ident16=ident16, poolm=poolm16,
    )

    in_maps = []
    for c in range(n_cores):
        m = dict(shared)
        m["x_sl"] = np.ascontiguousarray(
            x[c * NS:(c + 1) * NS]).astype(np.float16)
        gt = slice(c * T, (c + 1) * T)
        m["srcs"] = np.ascontiguousarray(srcs_p[gt].reshape(T * 128, K))
        m["dstf"] = np.ascontiguousarray(dstf_p[gt].reshape(T * 128, K))
        m["normv"] = np.ascontiguousarray(norm_p[gt].reshape(T * 128, K))
        m["nself"] = np.ascontiguousarray(
            nself[c * NS:(c + 1) * NS].reshape(T, 128).T)
        tpad = np.zeros((BL, 1024), np.int64)
        tpad[:, :1000] = tgt[c * BL:(c + 1) * BL]
        tl = tpad.reshape(BL, 8, 128)
        m["t_sb"] = np.ascontiguousarray(
            tl.transpose(2, 0, 1).reshape(128, BL * 8).astype(np.float16))
        in_maps.append(m)

    cfg = dict(n_cores=n_cores, N=N, NS=NS, T=T, BL=BL, K=K, GN=GN)
    return in_maps, cfg


def build(cfg, ablate=()):
    n_cores, NS, T, BL, K, GN = (cfg["n_cores"], cfg["NS"], cfg["T"],
                                 cfg["BL"], cfg["K"], cfg["GN"])
    N = cfg["N"]
    KT = K + 1          # +1 chunk for self loops
    ng = 128 // GN

    nc = bacc.Bacc(None, target_bir_lowering=False)

    def dinp(name, shape, dt=f32):
        return nc.dram_tensor(name, list(shape), dt, kind="ExternalInput")

    x_sl = dinp("x_sl", (NS, F), f16)
    srcs = dinp("srcs", (T * 128, K), i32)
    dstf = dinp("dstf", (T * 128, K), f16)
    normv = dinp("normv", (T * 128, K))
    nself_d = dinp("nself", (128, T))
    t_sb_d = dinp("t_sb", (128, BL * 8), f16)
    gatw_d = dinp("gatw", (F, HID), f16)
    asad_d = dinp("asad", (F, 2 * H), f16)
    gatb_d = dinp("gatb_rep", (128, HID))
    gcnw_d = dinp("gcnw", (128, NKK * HID), f16)
    gcnb_d = dinp("gcnb_rep", (128, HID))
    fcg1w_d = dinp("fcg1w", (128, 14 * 1500), f16)
    fcg1b_d = dinp("fcg1_bsw", (128, 12))
    fcg2w_d = dinp("fcg2w", (128, 12 * 128), f16)
    fcg2b_d = dinp("fcg2_bsw", (128, 1))
    w2_d = dinp("w2", (128, 8 * 256), f16)
    emb_d = dinp("emb", (26, 128), f16)
    fcxtw_d = dinp("fcxtw", (128, 32 * 128), f16)
    fcxtb_d = dinp("fcxt_bsw", (128, 1))
    fc1w_d = dinp("fc1w", (128, 2 * 1024), f16)
    fc1b_d = dinp("fc1_bsw", (128, 8))
    fc2w_d = dinp("fc2w", (128, 8 * 512), f16)
    fc2b_d = dinp("fc2_bsw", (128, 4))
    outw_d = dinp("outw", (128, 4), f16)
    outb_d = dinp("out_b", (1, 1))
    iota_d = dinp("iota", (128, K * 128), f16)
    iota26_d = dinp("iota26", (128, 8 * 26), f16)
    ident_d = dinp("ident16", (128, 128), f16)
    poolm_d = dinp("poolm", (128, ng), f16)

    outp = nc.dram_tensor("outp", [1, BL], f32, kind="ExternalOutput")

    htab_sl = nc.dram_tensor("htab_sl", [NS, FW], f8)
    htab = nc.dram_tensor("htab", [N, FW], f8, addr_space="Shared")
    g1_sl = nc.dram_tensor("g1_sl", [NS, HID], f8)
    g1tab = nc.dram_tensor("g1tab", [N, HID], f8, addr_space="Shared")
    gap_dram = nc.dram_tensor("gap_dram", [BL, HID], f16)

    FCH = [(kk * 128, min(128, HID - kk * 128)) for kk in range(NKK)]

    def tiles(n, step=128):
        return [(s, min(step, n - s)) for s in range(0, n, step)]

    with tile.TileContext(nc) as tc:
        with (
            tc.tile_pool(name="const", bufs=1) as cpool,
            tc.tile_pool(name="sb", bufs=2) as pool,
            tc.tile_pool(name="sm", bufs=6) as spool,
            tc.tile_pool(name="ps", bufs=2, space="PSUM") as psp,
            tc.tile_pool(name="pstr", bufs=2, space="PSUM") as pst,
            tc.tile_pool(name="psm", bufs=2, space="PSUM") as psm,
        ):
            # ---------- resident constants ----------
            # phase-A-critical consts on SP queue (first in line)
            def load_sp(name, dram, shape, dt=f32, rows=None):
                t_ = cpool.tile(list(shape), dt, tag=name, name=name)
                nc.sync.dma_start(out=t_[:rows, :] if rows else t_[:],
                                  in_=dram[:])
                return t_

            # heavy weights prefetched on the Pool SWDGE queue (idle early)
            def load_dve(name, dram, shape, dt=f16, rows=None):
                t_ = cpool.tile(list(shape), dt, tag=name, name=name)
                nc.gpsimd.dma_start(out=t_[:rows, :] if rows else t_[:],
                                    in_=dram[:])
                return t_

            ident_sb = load_sp("ident", ident_d, [128, 128], f16)
            gatw_sb = load_sp("gatw", gatw_d, [F, HID], f16)
            asad_sb = load_sp("asad", asad_d, [F, 2 * H], f16)
            iota_sb = load_sp("iota", iota_d, [128, K * 128], f16)
            gatb_sb = load_sp("gatb", gatb_d, [128, HID])
            nself_sb = load_sp("nself", nself_d, [128, T])

            gcnw_sb = load_dve("gcnw", gcnw_d, [128, NKK * HID])
            gcnb_sb = load_dve("gcnb", gcnb_d, [128, HID], f32)
            fcg1w_sb = load_dve("fcg1w", fcg1w_d, [128, 14 * 1500])
            fcg2w_sb = load_dve("fcg2w", fcg2w_d, [128, 12 * 128])
            w2_sb = load_dve("w2", w2_d, [128, 8 * 256])
            emb_sb = load_dve("emb", emb_d, [26, 128])
            fcxtw_sb = load_dve("fcxtw", fcxtw_d, [128, 32 * 128])
            fc1w_sb = load_dve("fc1w", fc1w_d, [128, 2 * 1024])
            fc2w_sb = load_dve("fc2w", fc2w_d, [128, 8 * 512])
            outw_sb = load_dve("outw", outw_d, [128, 4])
            iota26_sb = load_dve("iota26", iota26_d, [128, 8 * 26])
            t_sb = load_dve("tsb", t_sb_d, [128, BL * 8])
            poolm_sb = load_dve("poolm", poolm_d, [128, ng])
            fcg1b_sb = load_dve("fcg1b", fcg1b_d, [128, 12], f32)
            fcg2b_sb = load_dve("fcg2b", fcg2b_d, [128, 1], f32)
            fcxtb_sb = load_dve("fcxtb", fcxtb_d, [128, 1], f32)
            fc1b_sb = load_dve("fc1b", fc1b_d, [128, 8], f32)
            fc2b_sb = load_dve("fc2b", fc2b_d, [128, 4], f32)
            outb_sb = load_dve("outb", outb_d, [1, 1], f32)

            adst_sb = cpool.tile([128, T * H], f16, tag="adst")
            cvT_sb = cpool.tile([128, BL * 32], f16, tag="cvT")

            # ================= Phase A: h | a_src | a_dst ===============
            for t in range(T):
                rows = slice(t * 128, (t + 1) * 128)
                x_t = pool.tile([128, F], f16, tag="x_t")
                nc.sync.dma_start(out=x_t[:], in_=x_sl[rows, :])
                xt_ps = pst.tile([128, 128], f16, tag="tr")
                nc.tensor.transpose(out=xt_ps[:F, :], in_=x_t[:],
                                    identity=ident_sb[:])
                xT = pool.tile([F, 128], f16, tag="xT")
                nc.scalar.activation(out=xT[:], in_=xt_ps[:F, :], func=AF.Copy)
                h_ps = psp.tile([128, HID], f32, tag="big")
                nc.tensor.matmul(out=h_ps[:, :512], lhsT=xT[:],
                                 rhs=gatw_sb[:, :512], start=True, stop=True)
                nc.tensor.matmul(out=h_ps[:, 512:], lhsT=xT[:],
                                 rhs=gatw_sb[:, 512:], start=True, stop=True)
                asd_ps = psm.tile([128, 96], f32, tag="mlp")
                nc.tensor.matmul(out=asd_ps[:, :2 * H], lhsT=xT[:],
                                 rhs=asad_sb[:], start=True, stop=True)
                htile = pool.tile([128, HID], f8, tag="htile")
                nc.scalar.activation(out=htile[:], in_=h_ps[:], func=AF.Copy)
                ha8 = spool.tile([128, H], f8, tag="ha8")
                nc.vector.tensor_copy(out=ha8[:], in_=asd_ps[:, :H])
                nc.vector.tensor_copy(out=adst_sb[:, t * H:(t + 1) * H],
                                      in_=asd_ps[:, H:2 * H])
                nc.sync.dma_start(out=htab_sl[rows, :HID], in_=htile[:])
                nc.sync.dma_start(out=htab_sl[rows, HID:FW], in_=ha8[:])

            if "coll" not in ablate:
                nc.gpsimd.collective_compute(
                    "AllGather", OP.bypass,
                    replica_groups=[list(range(n_cores))],
                    ins=[htab_sl[:]], outs=[htab[:]],
                )
            else:
                nc.gpsimd.dma_start(out=htab[:NS, :], in_=htab_sl[:])

            # ---- protein branch conv (half 1) — overlaps AllGather 1 ----
            def conv_block(b_lo, b_hi):
                for b in range(b_lo, b_hi):
                    oh = pool.tile([128, 8 * 26], f16, tag="oh")
                    nc.vector.tensor_tensor(
                        out=oh[:].rearrange("p (k c) -> p k c", c=26),
                        in0=iota26_sb[:].rearrange("p (k c) -> p k c", c=26),
                        in1=t_sb[:, b * 8:(b + 1) * 8].unsqueeze(2)
                            .to_broadcast([128, 8, 26]),
                        op=OP.is_equal)
                    at_ps = psm.tile([26, 256], f32, tag="mlp")
                    for ic in range(8):
                        icn = min(128, 1000 - ic * 128)
                        nc.tensor.matmul(
                            out=at_ps[:],
                            lhsT=oh[:icn, ic * 26:(ic + 1) * 26],
                            rhs=w2_sb[:icn, ic * 256:(ic + 1) * 256],
                            start=(ic == 0), stop=(ic == 7))
                    at_sb = pool.tile([26, 256], f16, tag="at_sb")
                    nc.scalar.activation(out=at_sb[:], in_=at_ps[:],
                                         func=AF.Copy)
                    cv_ps = psm.tile([121, 32], f32, tag="mlp")
                    for k in range(8):
                        nc.tensor.matmul(out=cv_ps[:],
                                         lhsT=emb_sb[:, k:k + 121],
                                         rhs=at_sb[:, k * 32:(k + 1) * 32],
                                         start=(k == 0), stop=(k == 7))
                    nc.scalar.activation(out=cvT_sb[:121, b * 32:(b + 1) * 32],
                                         in_=cv_ps[:], func=AF.Copy)

            if "conv" not in ablate:
                conv_block(0, BL // 2)
            else:
                nc.gpsimd.memset(cvT_sb[:], 0.0)

            # ================= Phase B: GAT edge aggregation =============
            MD = (KT + 1) // 2 + 1      # m-multiply chunks on DVE; rest Pool
            for t in range(T):
                rows = slice(t * 128, (t + 1) * 128)
                sc = spool.tile([128, K], i32, tag="sc")
                nc.sync.dma_start(out=sc[:], in_=srcs[rows, :])
                df = spool.tile([128, K], f16, tag="df")
                nc.sync.dma_start(out=df[:], in_=dstf[rows, :])
                G = pool.tile([128, KT * FW], f8, tag="G")
                for c in range(K):
                    if "gather" in ablate:
                        nc.sync.dma_start(
                            out=G[:, c * FW:(c + 1) * FW],
                            in_=htab[t * 128:(t + 1) * 128, :])
                    else:
                        nc.gpsimd.indirect_dma_start(
                            out=G[:, c * FW:(c + 1) * FW], out_offset=None,
                            in_=htab[:],
                            in_offset=bass.IndirectOffsetOnAxis(
                                ap=sc[:, c:c + 1], axis=0),
                        )
                nc.sync.dma_start(out=G[:, K * FW:KT * FW],
                                  in_=htab_sl[rows, :])
                sel = pool.tile([128, K * 128], f16, tag="sel")
                nc.vector.tensor_tensor(
                    out=sel[:].rearrange("p (k d) -> p k d", d=128),
                    in0=iota_sb[:].rearrange("p (k d) -> p k d", d=128),
                    in1=df[:].unsqueeze(2).to_broadcast([128, K, 128]),
                    op=OP.is_equal)
                sm_ps = psm.tile([128, 96], f32, tag="mlp")
                for c in range(K):
                    st_ps = pst.tile([128, 128], f16, tag="tr")
                    nc.tensor.transpose(out=st_ps[:],
                                        in_=sel[:, c * 128:(c + 1) * 128],
                                        identity=ident_sb[:])
                    selT = pool.tile([128, 128], f16, tag=f"selT{c}")
                    nc.vector.tensor_copy(out=selT[:], in_=st_ps[:])
                    nc.tensor.matmul(out=sm_ps[:, c * H:(c + 1) * H],
                                     lhsT=selT[:],
                                     rhs=adst_sb[:, t * H:(t + 1) * H],
                                     start=True, stop=True)
                al = pool.tile([128, KT * H], f32, tag="al")
                nc.vector.tensor_tensor(
                    out=al[:, :K * H].rearrange("p (k h) -> p k h", h=H),
                    in0=G[:].rearrange("p (k w) -> p k w", w=FW)[:, :K, HID:FW],
                    in1=sm_ps[:, :K * H].rearrange("p (k h) -> p k h", h=H),
                    op=OP.add)
                nc.vector.tensor_tensor(
                    out=al[:, K * H:KT * H],
                    in0=G[:, K * FW + HID:K * FW + FW],
                    in1=adst_sb[:, t * H:(t + 1) * H],
                    op=OP.add)
                al2 = pool.tile([128, KT * H], f32, tag="al2")
                nc.vector.tensor_scalar(out=al2[:], in0=al[:], scalar1=0.2,
                                        scalar2=None, op0=OP.mult)
                nc.vector.tensor_tensor(out=al2[:], in0=al2[:], in1=al[:],
                                        op=OP.max)
                p16 = pool.tile([128, KT * H], f16, tag="p16")
                nc.scalar.activation(out=p16[:], in_=al2[:], func=AF.Exp)
                for c in range(K):
                    nc.tensor.matmul(out=sm_ps[:, 80:80 + H],
                                     lhsT=sel[:, c * 128:(c + 1) * 128],
                                     rhs=p16[:, c * H:(c + 1) * H],
                                     start=(c == 0), stop=(c == K - 1))
                dtot = spool.tile([128, H], f32, tag="dtot")
                nc.vector.tensor_tensor(out=dtot[:], in0=sm_ps[:, 80:80 + H],
                                        in1=p16[:, K * H:KT * H], op=OP.add)
                rd = spool.tile([128, H], f32, tag="rd")
                nc.vector.reciprocal(out=rd[:], in_=dtot[:])
                m = pool.tile([128, KT * HID], f16, tag="m")
                nc.vector.tensor_tensor(
                    out=m[:, :MD * HID].rearrange(
                        "p (k h r) -> p k h r", h=H, r=F),
                    in0=G[:].rearrange("p (k w) -> p k w", w=FW)[:, :MD, :HID]
                         .rearrange("p k (h r) -> p k h r", r=F),
                    in1=p16[:, :MD * H].rearrange("p (k h) -> p k h", h=H)
                         .unsqueeze(3).to_broadcast([128, MD, H, F]),
                    op=OP.mult)
                nc.gpsimd.tensor_tensor(
                    out=m[:, MD * HID:].rearrange(
                        "p (k h r) -> p k h r", h=H, r=F),
                    in0=G[:].rearrange("p (k w) -> p k w", w=FW)[:, MD:KT, :HID]
                         .rearrange("p k (h r) -> p k h r", r=F),
                    in1=p16[:, MD * H:].rearrange("p (k h) -> p k h", h=H)
                         .unsqueeze(3).to_broadcast([128, KT - MD, H, F]),
                    op=OP.mult)
                g1_ps = psp.tile([128, HID], f32, tag="big")
                for c in range(KT):
                    lhsT = (sel[:, c * 128:(c + 1) * 128] if c < K
                            else ident_sb[:])
                    nc.tensor.matmul(out=g1_ps[:, :512], lhsT=lhsT,
                                     rhs=m[:, c * HID: c * HID + 512],
                                     start=(c == 0), stop=(c == KT - 1))
                    nc.tensor.matmul(out=g1_ps[:, 512:], lhsT=lhsT,
                                     rhs=m[:, c * HID + 512:(c + 1) * HID],
                                     start=(c == 0), stop=(c == KT - 1))
                g1n = pool.tile([128, HID], f32, tag="g1n")
                nc.vector.tensor_tensor(
                    out=g1n[:].rearrange("p (h r) -> p h r", r=F),
                    in0=g1_ps[:].rearrange("p (h r) -> p h r", r=F),
                    in1=rd[:].unsqueeze(2).to_broadcast([128, H, F]),
                    op=OP.mult)
                nc.vector.tensor_tensor(out=g1n[:], in0=g1n[:], in1=gatb_sb[:],
                                        op=OP.add)
                g1t = pool.tile([128, HID], f8, tag="g1t")
                nc.scalar.activation(out=g1t[:], in_=g1n[:], func=AF.Relu)
                nc.sync.dma_start(out=g1_sl[rows, :], in_=g1t[:])

            if "coll" not in ablate:
                nc.gpsimd.collective_compute(
                    "AllGather", OP.bypass,
                    replica_groups=[list(range(n_cores))],
                    ins=[g1_sl[:]], outs=[g1tab[:]],
                )
            else:
                nc.gpsimd.dma_start(out=g1tab[:NS, :], in_=g1_sl[:])

            # ---- protein branch conv (half 2) — overlaps AllGather 2 ----
            if "conv" not in ablate:
                conv_block(BL // 2, BL)

            # ============ Phase D: GCN aggregation + z + pooling =========
            gmpT_sb = []
            gapT_sb = []
            for kk in range(NKK):
                gmpT_sb.append(cpool.tile([128, BL], f16, tag=f"gmpT{kk}",
                                          name=f"gmpT{kk}"))
                gapT_sb.append(cpool.tile([128, BL], f16, tag=f"gapT{kk}",
                                          name=f"gapT{kk}"))

            for t in range(T):
                rows = slice(t * 128, (t + 1) * 128)
                sc = spool.tile([128, K], i32, tag="sc")
                nc.sync.dma_start(out=sc[:], in_=srcs[rows, :])
                df = spool.tile([128, K], f16, tag="df")
                nc.sync.dma_start(out=df[:], in_=dstf[rows, :])
                nv = spool.tile([128, K], f32, tag="nv")
                nc.sync.dma_start(out=nv[:], in_=normv[rows, :])
                G2 = pool.tile([128, K * HID], f8, tag="G")
                for c in range(K):
                    if "gather" in ablate:
                        nc.sync.dma_start(
                            out=G2[:, c * HID:(c + 1) * HID],
                            in_=g1tab[t * 128:(t + 1) * 128, :])
                    else:
                        nc.gpsimd.indirect_dma_start(
                            out=G2[:, c * HID:(c + 1) * HID], out_offset=None,
                            in_=g1tab[:],
                            in_offset=bass.IndirectOffsetOnAxis(
                                ap=sc[:, c:c + 1], axis=0),
                        )
                g1loc = pool.tile([128, HID], f8, tag="g1loc")
                nc.sync.dma_start(out=g1loc[:], in_=g1_sl[rows, :])
                sel = pool.tile([128, K * 128], f16, tag="sel")
                nc.vector.tensor_tensor(
                    out=sel[:].rearrange("p (k d) -> p k d", d=128),
                    in0=iota_sb[:].rearrange("p (k d) -> p k d", d=128),
                    in1=df[:].unsqueeze(2).to_broadcast([128, K, 128]),
                    op=OP.is_equal)
                wsel = pool.tile([128, K * 128], f8, tag="wsel")
                nc.vector.tensor_tensor(
                    out=wsel[:].rearrange("p (k d) -> p k d", d=128),
                    in0=sel[:].rearrange("p (k d) -> p k d", d=128),
                    in1=nv[:].unsqueeze(2).to_broadcast([128, K, 128]),
                    op=OP.mult)
                agg_ps = psp.tile([128, HID], f32, tag="big")
                for c in range(K):
                    nc.tensor.matmul(out=agg_ps[:, :512],
                                     lhsT=wsel[:, c * 128:(c + 1) * 128],
                                     rhs=G2[:, c * HID: c * HID + 512],
                                     start=(c == 0), stop=(c == K - 1))
                    nc.tensor.matmul(out=agg_ps[:, 512:],
                                     lhsT=wsel[:, c * 128:(c + 1) * 128],
                                     rhs=G2[:, c * HID + 512:(c + 1) * HID],
                                     start=(c == 0), stop=(c == K - 1))
                t1 = pool.tile([128, HID], f16, tag="t1")
                nc.vector.tensor_scalar(out=t1[:], in0=g1loc[:],
                                        scalar1=nself_sb[:, t:t + 1],
                                        scalar2=None, op0=OP.mult)
                s16 = pool.tile([128, HID], f16, tag="s16")
                nc.vector.tensor_tensor(out=s16[:], in0=agg_ps[:], in1=t1[:],
                                        op=OP.add)
                z_ps = psp.tile([128, HID], f32, tag="big")
                for kk, (ks, kn) in enumerate(FCH):
                    sT_ps = pst.tile([128, 128], f16, tag="tr")
                    nc.tensor.transpose(out=sT_ps[:kn, :],
                                        in_=s16[:, ks:ks + kn],
                                        identity=ident_sb[:])
                    sT = pool.tile([128, 128], f16, tag=f"sT{kk}")
                    nc.vector.tensor_copy(out=sT[:kn, :], in_=sT_ps[:kn, :])
                    nc.tensor.matmul(
                        out=z_ps[:, :512], lhsT=sT[:kn, :],
                        rhs=gcnw_sb[:kn, kk * HID:kk * HID + 512],
                        start=(kk == 0), stop=(kk == NKK - 1))
                    nc.tensor.matmul(
                        out=z_ps[:, 512:], lhsT=sT[:kn, :],
                        rhs=gcnw_sb[:kn, kk * HID + 512:(kk + 1) * HID],
                        start=(kk == 0), stop=(kk == NKK - 1))
                g2a = pool.tile([128, HID], f32, tag="g2a")
                nc.vector.tensor_tensor(out=g2a[:], in0=z_ps[:],
                                        in1=gcnb_sb[:], op=OP.add)
                g2b = pool.tile([128, HID], f16, tag="g2b")
                nc.scalar.activation(out=g2b[:], in_=g2a[:], func=AF.Relu)
                gp_ps = psp.tile([128, HID], f32, tag="big", name="gp_ps")
                nc.tensor.matmul(out=gp_ps[:ng, :512],
                                 lhsT=poolm_sb[:, :ng],
                                 rhs=g2b[:, :512], start=True, stop=True)
                nc.tensor.matmul(out=gp_ps[:ng, 512:],
                                 lhsT=poolm_sb[:, :ng],
                                 rhs=g2b[:, 512:], start=True, stop=True)
                gtmp = pool.tile([ng, HID], f16, tag="gtmp")
                nc.scalar.activation(out=gtmp[:], in_=gp_ps[:ng, :],
                                     func=AF.Copy)
                nc.sync.dma_start(out=gap_dram[ng * t:ng * (t + 1), :],
                                  in_=gtmp[:])
                for kk, (ks, kn) in enumerate(FCH):
                    tp_ps = pst.tile([128, 128], f16, tag="tr")
                    nc.tensor.transpose(out=tp_ps[:kn, :],
                                        in_=g2b[:, ks:ks + kn],
                                        identity=ident_sb[:])
                    nc.vector.reduce_max(
                        out=gmpT_sb[kk][:kn, ng * t:ng * (t + 1)],
                        in_=tp_ps[:kn, :].rearrange("p (g n) -> p g n", n=GN),
                        axis=AX.X)

            gap_acc = cpool.tile([BL, HID], f16, tag="gap_acc")
            nc.sync.dma_start(out=gap_acc[:], in_=gap_dram[:])
            for kk, (ks, kn) in enumerate(FCH):
                gt_ps = pst.tile([128, 128], f16, tag="tr", name="gt_ps")
                nc.tensor.transpose(out=gt_ps[:kn, :BL],
                                    in_=gap_acc[:, ks:ks + kn],
                                    identity=ident_sb[:BL, :BL])
                nc.vector.tensor_copy(out=gapT_sb[kk][:kn, :],
                                      in_=gt_ps[:kn, :BL])

            # ================= Phase E: MLPs =================
            y1_sb = cpool.tile([128, 12 * BL], f16, tag="y1")
            for mi, (ms, mn) in enumerate(tiles(1500)):
                y_ps = psm.tile([128, BL], f32, tag="mlp")
                for kk, (ks, kn) in enumerate(FCH):
                    nc.tensor.matmul(
                        out=y_ps[:mn, :],
                        lhsT=fcg1w_sb[:kn, kk * 1500 + ms:kk * 1500 + ms + mn],
                        rhs=gmpT_sb[kk][:kn, :],
                        start=(kk == 0), stop=False)
                for kk, (ks, kn) in enumerate(FCH):
                    nc.tensor.matmul(
                        out=y_ps[:mn, :],
                        lhsT=fcg1w_sb[:kn,
                                      (NKK + kk) * 1500 + ms:
                                      (NKK + kk) * 1500 + ms + mn],
                        rhs=gapT_sb[kk][:kn, :],
                        start=False, stop=(kk == NKK - 1))
                nc.scalar.activation(out=y1_sb[:mn, mi * BL:(mi + 1) * BL],
                                     in_=y_ps[:mn, :], func=AF.Relu,
                                     bias=fcg1b_sb[:mn, mi:mi + 1])

            xc0 = cpool.tile([128, BL], f16, tag="xc0")
            y2_ps = psm.tile([128, BL], f32, tag="mlp")
            kt2 = tiles(1500)
            for kk, (ks, kn) in enumerate(kt2):
                nc.tensor.matmul(out=y2_ps[:],
                                 lhsT=fcg2w_sb[:kn, kk * 128:(kk + 1) * 128],
                                 rhs=y1_sb[:kn, kk * BL:(kk + 1) * BL],
                                 start=(kk == 0), stop=(kk == len(kt2) - 1))
            nc.vector.tensor_scalar(out=xc0[:], in0=y2_ps[:],
                                    scalar1=fcg2b_sb[:, 0:1], scalar2=None,
                                    op0=OP.add)

            xc1 = cpool.tile([128, BL], f16, tag="xc1")
            xt_ps = psm.tile([128, BL], f32, tag="mlp")
            for o in range(32):
                nc.tensor.matmul(
                    out=xt_ps[:],
                    lhsT=fcxtw_sb[:121, o * 128:(o + 1) * 128],
                    rhs=cvT_sb[:121, :].rearrange("p (b o) -> p b o",
                                                  o=32)[:, :, o],
                    start=(o == 0), stop=(o == 31))
            nc.vector.tensor_scalar(out=xc1[:], in0=xt_ps[:],
                                    scalar1=fcxtb_sb[:, 0:1], scalar2=None,
                                    op0=OP.add)

            # ---- head ----
            y3_sb = cpool.tile([128, 8 * BL], f16, tag="y3")
            for mi in range(8):
                y_ps = psm.tile([128, BL], f32, tag="mlp")
                for kk in range(2):
                    rhs = xc0 if kk == 0 else xc1
                    nc.tensor.matmul(
                        out=y_ps[:],
                        lhsT=fc1w_sb[:, kk * 1024 + mi * 128:
                                     kk * 1024 + (mi + 1) * 128],
                        rhs=rhs[:], start=(kk == 0), stop=(kk == 1))
                nc.scalar.activation(out=y3_sb[:, mi * BL:(mi + 1) * BL],
                                     in_=y_ps[:], func=AF.Relu,
                                     bias=fc1b_sb[:, mi:mi + 1])
            y4_sb = cpool.tile([128, 4 * BL], f16, tag="y4")
            for mi in range(4):
                y_ps = psm.tile([128, BL], f32, tag="mlp")
                for kk in range(8):
                    nc.tensor.matmul(
                        out=y_ps[:],
                        lhsT=fc2w_sb[:, kk * 512 + mi * 128:
                                     kk * 512 + (mi + 1) * 128],
                        rhs=y3_sb[:, kk * BL:(kk + 1) * BL],
                        start=(kk == 0), stop=(kk == 7))
                nc.scalar.activation(out=y4_sb[:, mi * BL:(mi + 1) * BL],
                                     in_=y_ps[:], func=AF.Relu,
                                     bias=fc2b_sb[:, mi:mi + 1])
            o_ps = psm.tile([1, BL], f32, tag="mlp")
            for kk in range(4):
                nc.tensor.matmul(out=o_ps[:], lhsT=outw_sb[:, kk:kk + 1],
                                 rhs=y4_sb[:, kk * BL:(kk + 1) * BL],
                                 start=(kk == 0), stop=(kk == 3))
            o_sb = cpool.tile([1, BL], f32, tag="o_sb")
            nc.vector.tensor_scalar(out=o_sb[:], in0=o_ps[:],
                                    scalar1=outb_sb[:, 0:1], scalar2=None,
                                    op0=OP.add)
            nc.sync.dma_start(out=outp[:], in_=o_sb[:])

    nc.finalize()
    return nc


def run(inp, n_cores=8, trace=False):
    from concourse.bass_utils import run_bass_kernel_spmd
    in_maps, cfg = host_prep(inp, n_cores)
    nc = build(cfg)
    res = run_bass_kernel_spmd(
        nc, in_maps, list(range(n_cores)), trace=trace,
        trace_cores=list(range(n_cores)) if trace else None)
    out = np.concatenate(
        [res.results[c]["outp"].reshape(-1, 1) for c in range(n_cores)], 0)
    return out, res


_CACHED = {}


def kernel(**inputs):
    """Full-input entry point: shards across 8 NeuronCores internally."""
    n_cores = 8
    in_maps, cfg = host_prep(inputs, n_cores)
    key = (cfg["N"], cfg["T"], cfg["BL"], cfg["K"], cfg["GN"])
    nc = _CACHED.get(key)
    if nc is None:
        nc = build(cfg)
        _CACHED[key] = nc
    from concourse.bass_utils import run_bass_kernel_spmd
    res = run_bass_kernel_spmd(nc, in_maps, list(range(n_cores)))
    out = np.concatenate(
        [res.results[c]["outp"].reshape(-1, 1) for c in range(n_cores)], 0)
    return out.astype(np.float32)
